# revision 28
# baseline (speedup 1.0000x reference)
"""Trainium2 Bass kernel for the DiagSGP particle update.

Math (per particle n, with m=64 inducing points, p=32 obs dims, pm=2048):
    Kfz = var*exp(-||x-z||^2/(2 ls^2))            (n, m)
    A   = Kfz @ Kzz^-1                            (n, m)
    B   = var - sum(Kfz*A, -1)                    (n,)
    c   = B*diag(K) + noise_var                   (n, p)
    G   = (gamma reshaped (n,p,m)) . A^2          (n, p)
    d   = G @ (K*K)^T + c                         (n, p)
    S   = (1/d) @ (K*K)                           (n, p)
    W   = (y/c) @ K                               (n, p)
    u   = gamma * A^2_e * S_e   (e = Kronecker expansion to (n, pm))
    g   = gamma * (1 - u)
    m_new = (1 - u) * (z + gamma * A_e * W_e)

Sharding: data-parallel over particles, 64 per core on 8 cores.  The
shared 64x64 Kzz factorization is precomputed on host (O(m^3), particle
independent); everything that scales with n runs on device.

On-device layout: the (64, 2048) per-core tensors are stored as
(128, 1024): partition = kh*64 + n (kh in {0,1} = front/back half of the
Kronecker axis), free = 1024.  Per-particle small tensors (A, S, W) are
materialized in duplicated/split form so every Kronecker expansion is a
stride-0 broadcast access pattern - no big materialization.  All small
constants ship in one packed (128, 544) DMA; gamma/z/outputs ride
separate DMA queues (sync vs gpsimd) to avoid queue serialization.
"""

from contextlib import ExitStack

import numpy as np

_N, _Q, _P, _M = 512, 8, 32, 64
_PM = _P * _M          # 2048
_NCORES = 8
_NS = _N // _NCORES    # 64 particles per core
_JITTER = 1e-5
_HALF = _PM // 2       # 1024
_J = _P // 2           # 16 j-groups per partition row

# Packed-constant column offsets within the (128, 544) tile.
_C_ID2, _C_KINV, _C_XA, _C_IA, _C_YT = 0, 128, 192, 256, 320
_C_KRHS, _C_KSQ, _C_DLA, _C_DLB, _C_CL = 384, 416, 448, 480, 512
_C_W = 544

# Fuse (1-u) into the g/m products via reverse0 scalar_tensor_tensor.
# Hardware ISA supports it; CoreSim does not (use False when simulating).
STT_REV = False

# Set by test harness to request an NTFF-profiled run; results stashed below.
TRACE = False
LAST_EXEC_NS = None
LAST_RESULTS = None

_module_cache = {}


def _build_module(var: float):
    import concourse.mybir as mybir
    import concourse.tile as tile
    from concourse import bacc

    f32 = mybir.dt.float32
    AF = mybir.ActivationFunctionType
    AL = mybir.AluOpType
    AX = mybir.AxisListType

    nc = bacc.Bacc("TRN2", debug=False, enable_asserts=False)

    d_c = nc.dram_tensor("consts", (128, _C_W), f32, kind="ExternalInput").ap()
    d_gam = nc.dram_tensor("gam", (128, _HALF), f32, kind="ExternalInput").ap()
    d_z = nc.dram_tensor("zz", (128, _HALF), f32, kind="ExternalInput").ap()
    d_go = nc.dram_tensor("g_out", (128, _HALF), f32, kind="ExternalOutput").ap()
    d_mo = nc.dram_tensor("m_out", (128, _HALF), f32, kind="ExternalOutput").ap()

    def stt(out, in0, scalar, in1, op0, op1, reverse0=False, accum_out=None):
        """scalar_tensor_tensor with reverse0: out = (scalar op0 in0) op1 in1."""
        eng = nc.vector
        outs = [eng.lower_ap(out)]
        if accum_out is not None:
            outs.append(eng.lower_ap(accum_out))
        return eng.add_instruction(
            mybir.InstTensorScalarPtr(
                name=nc.get_next_instruction_name(),
                is_scalar_tensor_tensor=True,
                op0=op0, reverse0=reverse0, op1=op1,
                ins=[eng.lower_ap(in0), eng.lower_ap_or_imm(scalar),
                     eng.lower_ap(in1)],
                outs=outs,
            ))

    with tile.TileContext(nc) as tc, ExitStack() as ctx:
        const = ctx.enter_context(tc.tile_pool(name="const", bufs=1))
        big = ctx.enter_context(tc.tile_pool(name="big", bufs=1))
        pp = ctx.enter_context(tc.tile_pool(name="psum", bufs=1, space="PSUM"))

        # Warm the ACT exp table set before any data arrives.
        warm = const.tile([1, 2], f32)
        nc.gpsimd.memset(warm[:, 0:1], 0.0)
        nc.scalar.activation(warm[:, 1:2], warm[:, 0:1], AF.Exp)

        # ---- input DMAs: consts then gamma on sync queue, z on gpsimd ----
        t_c = const.tile([128, _C_W], f32)
        nc.sync.dma_start(t_c[:], d_c)
        t_gam = big.tile([128, _HALF], f32)
        nc.sync.dma_start(t_gam[:], d_gam)
        t_z = big.tile([128, _HALF], f32)
        nc.gpsimd.dma_start(t_z[:], d_z)

        c_id2 = t_c[:, _C_ID2:_C_ID2 + 128]
        c_kinv = t_c[0:_M, _C_KINV:_C_KINV + _M]
        c_xa = t_c[0:_Q + 2, _C_XA:_C_XA + _NS]
        c_ia = t_c[0:_Q + 2, _C_IA:_C_IA + _M]
        c_yt = t_c[0:_P, _C_YT:_C_YT + _NS]
        c_krhs = t_c[0:_P, _C_KRHS:_C_KRHS + _P]
        c_ksq = t_c[0:_P, _C_KSQ:_C_KSQ + _P]
        c_dla = t_c[0:_J, _C_DLA:_C_DLA + _P]
        c_dlb = t_c[0:_J, _C_DLB:_C_DLB + _P]
        c_clhs = t_c[0:2, _C_CL:_C_CL + _P]

        # ---- small phase A: Kfz, A, B ----
        # One PSUM bank holds pre (cols 0:64) and pre^T (cols 64:128).
        ps_mm = pp.tile([_NS, 2 * _M], f32)
        nc.tensor.matmul(ps_mm[:, _M:2 * _M], c_ia, c_xa, start=True, stop=True)
        nc.tensor.matmul(ps_mm[:, 0:_M], c_xa, c_ia, start=True, stop=True)

        # Kfz^T duplicated along free so one matmul yields A on 128 partitions.
        t_kfzt2 = const.tile([_M, 2 * _NS], f32)
        nc.scalar.activation(t_kfzt2[:, 0:_NS], ps_mm[:, _M:2 * _M], AF.Exp)
        nc.scalar.activation(t_kfzt2[:, _NS:2 * _NS], ps_mm[:, _M:2 * _M],
                             AF.Exp)
        t_kfz = const.tile([_NS, _M], f32)
        nc.scalar.activation(t_kfz[:], ps_mm[:, 0:_M], AF.Exp)

        ps_a = pp.tile([128, _M], f32)
        nc.tensor.matmul(ps_a[:], t_kfzt2[:], c_kinv, start=True, stop=True)

        # Bsum[n] = sum_m Kfz*A; (the KA product itself is scratch)
        t_ka = const.tile([_NS, _M], f32)
        t_bsum = const.tile([_NS, 1], f32)
        stt(t_ka[:], t_kfz[:], 1.0, ps_a[0:_NS, :], AL.bypass, AL.mult,
            accum_out=t_bsum[:])

        # Bsum^T as a (1, 64) row, then bvo = [var - Bsum; ones] (2, 64).
        ps_tpb = pp.tile([1, _NS], f32)
        nc.tensor.transpose(ps_tpb[:], t_bsum[:], c_id2[0:_M, 0:_M])
        t_bvo = const.tile([2, _NS], f32)
        nc.vector.memset(t_bvo[:], 1.0)
        nc.scalar.activation(t_bvo[0:1, :], ps_tpb[:],
                             AF.Copy, bias=float(var), scale=-1.0)

        # ---- c path (independent of gamma): cT, ycT, W2 ----
        ps_cb = pp.tile([_P, _NS], f32)
        nc.tensor.matmul(ps_cb[:], c_clhs, t_bvo[:], start=True, stop=True)
        t_invct = const.tile([_P, _NS], f32)
        nc.vector.reciprocal(t_invct[:], ps_cb[:])
        t_yct = const.tile([_P, _NS], f32)
        nc.vector.tensor_tensor(t_yct[:], c_yt, t_invct[:], AL.mult)
        ps_w2 = pp.tile([128, _J], f32)
        nc.tensor.matmul(ps_w2[0:_NS, :], t_yct[:], c_krhs[:, 0:_J],
                         start=True, stop=True)
        nc.tensor.matmul(ps_w2[_NS:128, :], t_yct[:], c_krhs[:, _J:_P],
                         start=True, stop=True)

        # ---- big phase: s, t, G, then r3/r4 while the d/S chain runs ----
        gam3 = t_gam[:].rearrange("p (j m) -> p j m", j=_J)
        a_bc = ps_a[:].unsqueeze(1).broadcast_to([128, _J, _M])

        t_s = big.tile([128, _HALF], f32)
        s3 = t_s[:].rearrange("p (j m) -> p j m", j=_J)
        nc.vector.tensor_tensor(s3, gam3, a_bc, AL.mult)
        t_t = big.tile([128, _HALF], f32)
        t3 = t_t[:].rearrange("p (j m) -> p j m", j=_J)
        nc.vector.tensor_tensor(t3, s3, a_bc, AL.mult)
        t_gs = const.tile([128, _J], f32)
        nc.vector.tensor_reduce(t_gs[:], t3, axis=AX.X, op=AL.add)

        w2_bc = ps_w2[:].unsqueeze(2).broadcast_to([128, _J, _M])
        t_r3 = big.tile([128, _HALF], f32)
        r33 = t_r3[:].rearrange("p (j m) -> p j m", j=_J)
        nc.vector.tensor_tensor(r33, s3, w2_bc, AL.mult)
        t_r4 = big.tile([128, _HALF], f32)
        nc.vector.tensor_tensor(t_r4[:], t_z[:], t_r3[:], AL.add)

        # ---- small phase B (overlaps r3/r4 above): G^T, d, S ----
        # One PE transpose: Gs (128, 16) -> GsT (16, 128); column kh*64+n
        # holds G[n, kh*16 + j'].
        ps_tp = pp.tile([_J, 128], f32)
        nc.tensor.transpose(ps_tp[:], t_gs[:], c_id2)
        t_rhse = const.tile([_J, 128], f32)
        nc.scalar.activation(t_rhse[:], ps_tp[:], AF.Copy)

        # d^T accumulated from the two kh halves of G plus [diag K; nv].[B; 1].
        ps_dt = pp.tile([_P, _NS], f32)
        nc.tensor.matmul(ps_dt[:], c_dla, t_rhse[:, 0:_NS],
                         start=True, stop=False)
        nc.tensor.matmul(ps_dt[:], c_dlb, t_rhse[:, _NS:128],
                         start=False, stop=False)
        nc.tensor.matmul(ps_dt[:], c_clhs, t_bvo[:], start=False, stop=True)
        t_invdt = const.tile([_P, _NS], f32)
        nc.vector.reciprocal(t_invdt[:], ps_dt[:])
        ps_s2 = pp.tile([128, _J], f32)
        nc.tensor.matmul(ps_s2[0:_NS, :], t_invdt[:], c_ksq[:, 0:_J],
                         start=True, stop=True)
        nc.tensor.matmul(ps_s2[_NS:128, :], t_invdt[:], c_ksq[:, _J:_P],
                         start=True, stop=True)

        # ---- big phase tail: u, then fused g = gam*(1-u), m = (1-u)*r4 ----
        s2_bc = ps_s2[:].unsqueeze(2).broadcast_to([128, _J, _M])
        t_u = big.tile([128, _HALF], f32)
        u3 = t_u[:].rearrange("p (j m) -> p j m", j=_J)
        nc.vector.tensor_tensor(u3, t3, s2_bc, AL.mult)

        t_g = big.tile([128, _HALF], f32)
        t_m = big.tile([128, _HALF], f32)
        if STT_REV:
            stt(t_g[:], t_u[:], 1.0, t_gam[:], AL.subtract, AL.mult,
                reverse0=True)
            nc.gpsimd.dma_start(d_go, t_g[:])
            stt(t_m[:], t_u[:], 1.0, t_r4[:], AL.subtract, AL.mult,
                reverse0=True)
        else:
            t_v = big.tile([128, _HALF], f32)
            nc.vector.tensor_scalar(t_v[:], t_u[:], -1.0, 1.0,
                                    AL.mult, AL.add)
            nc.vector.tensor_tensor(t_g[:], t_gam[:], t_v[:], AL.mult)
            nc.gpsimd.dma_start(d_go, t_g[:])
            nc.vector.tensor_tensor(t_m[:], t_v[:], t_r4[:], AL.mult)
        nc.sync.dma_start(d_mo, t_m[:])

    nc.compile()
    return nc


def _get_module(var: float):
    key = (round(float(var), 9), STT_REV)
    if key not in _module_cache:
        _module_cache[key] = _build_module(float(var))
    return _module_cache[key]


def _host_prep(x, y, z, gamma, inducing, K, var, lengthscale, noise_var):
    f32 = np.float32
    x = np.asarray(x, f32)
    y = np.asarray(y, f32)
    z2 = np.asarray(z, f32).reshape(_N, _PM)
    gam2 = np.asarray(gamma, f32).reshape(_N, _PM)
    inducing = np.asarray(inducing, f32)
    K = np.asarray(K, f32)
    var_f = float(var)
    ls2 = float(lengthscale) ** 2
    nv = float(noise_var)

    # Shared small factor: Kzz^-1 (64x64, particle independent).
    diff2 = ((inducing[:, None, :] - inducing[None, :, :]) ** 2).sum(-1)
    Kzz = var_f * np.exp(-0.5 * diff2 / ls2) + _JITTER * var_f * np.eye(_M)
    Kinv = np.linalg.inv(Kzz.astype(np.float64)).astype(f32)

    Ksq = np.ascontiguousarray(K * K)
    consts = np.zeros((128, _C_W), f32)
    consts[0:128, _C_ID2:_C_ID2 + 128] = np.eye(128, dtype=f32)
    consts[0:_M, _C_KINV:_C_KINV + _M] = Kinv
    consts[0:_Q, _C_IA:_C_IA + _M] = inducing.T / ls2
    consts[_Q, _C_IA:_C_IA + _M] = -0.5 * (inducing ** 2).sum(1) / ls2
    consts[_Q + 1, _C_IA:_C_IA + _M] = 1.0
    consts[0:_P, _C_KRHS:_C_KRHS + _P] = K
    consts[0:_P, _C_KSQ:_C_KSQ + _P] = Ksq
    consts[0:_J, _C_DLA:_C_DLA + _P] = Ksq.T[0:_J]
    consts[0:_J, _C_DLB:_C_DLB + _P] = Ksq.T[_J:_P]
    consts[0, _C_CL:_C_CL + _P] = np.diag(K)
    consts[1, _C_CL:_C_CL + _P] = nv

    in_maps = []
    for c in range(_NCORES):
        sl = slice(c * _NS, (c + 1) * _NS)
        cc = consts.copy()
        cc[0:_Q, _C_XA:_C_XA + _NS] = x[sl].T
        cc[_Q, _C_XA:_C_XA + _NS] = 1.0
        cc[_Q + 1, _C_XA:_C_XA + _NS] = (
            -0.5 * (x[sl] ** 2).sum(1) / ls2 + np.log(var_f))
        cc[0:_P, _C_YT:_C_YT + _NS] = y[sl].T
        gam_t = np.concatenate([gam2[sl, 0:_HALF], gam2[sl, _HALF:_PM]], 0)
        z_t = np.concatenate([z2[sl, 0:_HALF], z2[sl, _HALF:_PM]], 0)
        in_maps.append(dict(consts=cc,
                            gam=np.ascontiguousarray(gam_t),
                            zz=np.ascontiguousarray(z_t)))
    return in_maps, var_f


def kernel(x, y, z, gamma, inducing, K, var, lengthscale, noise_var):
    global LAST_EXEC_NS, LAST_RESULTS
    from concourse import bass_utils

    in_maps, var_f = _host_prep(x, y, z, gamma, inducing, K, var,
                                lengthscale, noise_var)
    nc = _get_module(var_f)
    res = bass_utils.run_bass_kernel_spmd(
        nc, in_maps, core_ids=list(range(_NCORES)), trace=TRACE)
    LAST_EXEC_NS = res.exec_time_ns
    LAST_RESULTS = res

    f32 = np.float32
    m_new = np.empty((_N, _PM), f32)
    g = np.empty((_N, _PM), f32)
    for c, r in enumerate(res.results):
        sl = slice(c * _NS, (c + 1) * _NS)
        go, mo = r["g_out"], r["m_out"]
        g[sl, 0:_HALF] = go[0:_NS]
        g[sl, _HALF:_PM] = go[_NS:128]
        m_new[sl, 0:_HALF] = mo[0:_NS]
        m_new[sl, _HALF:_PM] = mo[_NS:128]
    return (m_new[..., None], g[..., None])
